# revision 28
# baseline (speedup 1.0000x reference)
"""Trainium2 Bass kernel for nn_BasicNet (CondConv 3-branch + BN + channel shuffle).

v10 design (~180-195us HW, from 320us v3 baseline):
  - col-tiled unit pairs: units (even sample, odd sample) of the same branch
    run concurrently on PE col halves (PSUM partitions 0:63 / 64:127) —
    verified concurrent on HW (2nd MM of each pair shows ~0ns in trace).
  - tap-outer conv loops over 7 PSUM banks (one per 8-row tile) so the PE
    stream stays dense (HAM warm) and LDWEIGHTS amortizes over 7 matmuls.
  - all K=64 single taps served from the unshifted lower half: a start=False
    matmul with inputs at partition base 64 faults this HW (empirical).
  - pooling split across engines (unit A: DVE tensor_scalar+accum_out,
    unit B: ACT Copy+accum_out); Sum(x) rides the ACT evacuation accum_out;
    Sum(x^2) via DVE stt(x*1*x)+accum_out.  All 1x — DVE 2x/4x modes do not
    lower in this build.  Aggregation in bf16 on DVE.
  - one AllReduce at pipeline end of a folded [64, 6] per-core stat blob
    (AllGather returns garbage here; split/overlapped ARs serialize and
    block the gpsimd queue — both tried and reverted).
  - s0 passthrough is host-side unshard glue (pure memcpy); device output is
    compact g-major [NS, 6, 32, HW] so each unit's store is one contiguous
    800KB DMA (strided dests measured ~6x slower); host reorders channels.
  - stores alternate sync/scalar queues, overlapped with ACT/DVE normalize.
  - latency-critical small DMAs (att gather, stat folds, scale/bias dup) on
    the scalar HWDGE ring: the sync ring is FIFO and a small DMA queued
    behind bulk input loads waits ~15us for their transfers.
"""

import sys

if '/opt/trn_rl_repo' not in sys.path:
    sys.path.insert(0, '/opt/trn_rl_repo')

import numpy as np
import ml_dtypes

import concourse.bass as bass
import concourse.bacc as bacc
import concourse.tile as tile
from concourse import mybir
from concourse import bass_utils

F32 = mybir.dt.float32
BF16 = mybir.dt.bfloat16

N_CORES = 8
NS = 4                   # samples per core
H = W = 56
HW = H * W               # 3136
C = 64                   # channels per branch (Cin == O == 64)
KEXP = 4                 # CondConv experts
RT = 8                   # rows per conv tile
NT = RT * W              # 448 free elements per matmul tile
N_TILES = H // RT        # 7
M_TOTAL = 32 * HW        # BN stat count
EPS = 1e-5
ROW_SLACK = 64           # extra zero elements per channel row (>= max shift)
FLAT_MAX = 58 * 58       # largest padded image (sq)

# branch geometry.  For each branch the SBUF input tile holds the padded
# image on partitions 0:64 and the image shifted by `shift` elements on
# partitions 64:128.  K=128 'pair' matmuls contract tap (dy,dx) [lower] and
# the tap at flat offset +shift [upper] together.  K=64 'single' matmuls run
# on one row strip: 'lo' reads the unshifted half, 'hi' reads the shifted
# half (tap offset - shift must stay inside the padded row).
#   pairs:   list of base taps (dy, dx); partner tap = flat offset + shift
#   singles: list of (dy, dx, half) with half in {'lo', 'hi'}
BR = [
    # NOTE: 'hi'-served singles are disabled — a start=False matmul whose
    # inputs sit at partition base 64 faults the HW (isolated empirically).
    ('sq', (58, 58), 1, [(0, 0), (1, 0), (2, 0)],
     [(0, 2, 'lo'), (1, 2, 'lo'), (2, 2, 'lo')]),
    ('v', (58, 56), 56, [(0, 0)], [(2, 0, 'lo')]),
    ('h', (56, 58), 1, [(0, 0)], [(0, 2, 'lo')]),
]
# singles grouped into concurrent slots (row-strip packed)
BR_SLOTS = {
    'sq': [[0], [1], [2], [3], [4], [5]],   # 3 pairs + 3 singles
    'v': [[0], [1]],
    'h': [[0], [1]],
}

# pair order: (branch, (even sample, odd sample)) interleaved for balance
PAIRS = [(0, 0), (1, 0), (2, 0), (0, 1), (1, 1), (2, 1)]


def _col_taps(bi):
    """Per weight-column j: (kind, tap, half) where kind in {'pair','single'}."""
    bn, (ph, pw), shift, pairs, singles = BR[bi]
    cols = []
    for (dy, dx) in pairs:
        cols.append(('pair', (dy, dx), None))
    for (dy, dx, half) in singles:
        cols.append(('single', (dy, dx), half))
    return cols


def _build_nc():
    nc = bacc.Bacc('TRN2', target_bir_lowering=False, debug=False,
                   num_devices=N_CORES)

    xp = {}
    w_t = {}
    for bi, (bn, (ph, pw), shift, pairs, singles) in enumerate(BR):
        xp[bi] = nc.dram_tensor(f'xp_{bn}', [NS, C, ph * pw + ROW_SLACK], BF16,
                                kind='ExternalInput').ap()
        ncol = len(pairs) + len(singles)
        w_t[bi] = nc.dram_tensor(f'w_{bn}', [128, KEXP, ncol * C], BF16,
                                 kind='ExternalInput').ap()
    att_w = nc.dram_tensor('att_w', [128, 3, KEXP], F32, kind='ExternalInput').ap()
    att_b = nc.dram_tensor('att_b', [KEXP, 3], F32, kind='ExternalInput').ap()
    gb = nc.dram_tensor('gb', [C, 2, 3], F32, kind='ExternalInput').ap()
    # compact output: (n, g', c2, hw) with real channel = c2*8 + (2 + g');
    # g-major so each unit's store is one contiguous 800KB block
    out = nc.dram_tensor('out', [NS, 6, 32, HW], F32,
                         kind='ExternalOutput').ap()

    with tile.TileContext(nc) as tc:
        _emit(tc, xp, w_t, att_w, att_b, gb, out)

    nc.compile()
    return nc


def _emit(tc, xp, w_t, att_w, att_b, gb, out):
    nc = tc.nc
    from contextlib import ExitStack
    ctx = ExitStack()
    with ctx:
        persist = ctx.enter_context(tc.tile_pool(name='persist', bufs=1))
        inpool = ctx.enter_context(tc.tile_pool(name='inpool', bufs=6))
        aggp = ctx.enter_context(tc.tile_pool(name='aggp', bufs=4))
        smalls = ctx.enter_context(tc.tile_pool(name='smalls', bufs=4))
        pscrp = ctx.enter_context(tc.tile_pool(name='pscrp', bufs=2))
        pqpool = ctx.enter_context(tc.tile_pool(name='pqpool', bufs=4))
        sqscrp = ctx.enter_context(tc.tile_pool(name='sqscrp', bufs=2))
        bouncep = ctx.enter_context(tc.tile_pool(name='bouncep', bufs=4))
        psum_conv = ctx.enter_context(
            tc.tile_pool(name='psum_conv', bufs=7, space='PSUM'))
        psum_att = ctx.enter_context(
            tc.tile_pool(name='psum_att', bufs=1, space='PSUM'))
        dram = ctx.enter_context(tc.tile_pool(name='dram', bufs=1, space='DRAM'))

        # ---------- persistent SBUF state ----------
        w_sb = {}
        for bi, (bn, _, _, pairs, singles) in enumerate(BR):
            ncol = len(pairs) + len(singles)
            t = persist.tile([128, KEXP, ncol * C], BF16, tag=f'w_sb_{bi}',
                             name=f'w_sb_{bi}')
            nc.scalar.dma_start(out=t, in_=w_t[bi])
            w_sb[bi] = t
        att_w_sb = persist.tile([128, 3, KEXP], F32, tag='att_w_sb')
        nc.scalar.dma_start(out=att_w_sb, in_=att_w)
        att_b_sb = persist.tile([KEXP, 3], F32, tag='att_b_sb')
        nc.scalar.dma_start(out=att_b_sb, in_=att_b)
        gb_sb = persist.tile([C, 2, 3], F32, tag='gb_sb')
        nc.scalar.dma_start(out=gb_sb, in_=gb)

        # conv outputs (bf16): one [128, HW] tile per pair
        out_tiles = [persist.tile([128, HW], BF16, tag=f'out_{i}',
                                  name=f'out_{i}') for i in range(6)]
        # per-pair per-bank stat partials and the per-core stage
        sx_t = [persist.tile([128, N_TILES], F32, tag=f'sx_{i}',
                             name=f'sx_{i}') for i in range(6)]
        sxx_t = [persist.tile([128, N_TILES], F32, tag=f'sxx_{i}',
                              name=f'sxx_{i}') for i in range(6)]
        stage = persist.tile([128, 3, 2, 2], F32, tag='stage')  # (c,b,p,stat)

        cc_in = [dram.tile([64, 6], F32, name=f'cc_in_{i}')
                 for i in range(2)]
        cc_out = [dram.tile([64, 6], F32, name=f'cc_out_{i}')
                  for i in range(2)]
        g_sb = [persist.tile([64, 3, 2], F32, tag=f'g_{i}', name=f'g_{i}')
                for i in range(2)]

        # unit (s, bi) stores to g' in {2bi, 2bi+1}; src partition p maps
        # row-major onto (g'-2bi, c2) -- contiguous dest block
        ov = out

        in_tiles = {}   # (pair_idx, unit) -> tile
        pq_tiles = {}   # (pair_idx, unit) -> [2 half-images x 64ch] pool view

        def load_pair(p):
            bi, sp = PAIRS[p]
            bn, (ph, pw), shift, pairs, singles = BR[bi]
            flat = ph * pw
            hf = flat // 2
            ts = []
            for u in range(2):
                t = inpool.tile([128, FLAT_MAX], BF16, tag='in',
                                name=f'in_{p}_{u}')
                ts.append(t)
                in_tiles[(p, u)] = t
                q = pqpool.tile([128, FLAT_MAX // 2], BF16, tag='pq',
                                name=f'pq_{p}_{u}')
                pq_tiles[(p, u)] = q
            for u in range(2):   # pool views first: pools only need these
                xps = xp[bi][2 * sp + u]
                nc.sync.dma_start(out=pq_tiles[(p, u)][0:64, 0:hf],
                                  in_=xps[:, 0:hf])
                nc.sync.dma_start(out=pq_tiles[(p, u)][64:128, 0:hf],
                                  in_=xps[:, hf:flat])
            for u in range(2):
                xps = xp[bi][2 * sp + u]
                nc.sync.dma_start(out=ts[u][0:64, 0:flat], in_=xps[:, 0:flat])
            for u in range(2):
                xps = xp[bi][2 * sp + u]
                nc.sync.dma_start(out=ts[u][64:128, 0:flat],
                                  in_=xps[:, shift:shift + flat])

        def pool_att(p):
            """pool -> att matmul -> sigmoid -> gpsimd gather+broadcast."""
            bi, sp = PAIRS[p]
            bn, (ph, pw), shift, pairs, singles = BR[bi]
            flat = ph * pw
            hf = flat // 2
            pooled = smalls.tile([128, 2], F32, tag='pooled',
                                 name=f'pooled_{p}')
            for u in range(2):
                q = pq_tiles[(p, u)]
                pscr = pscrp.tile([128, FLAT_MAX], BF16, tag='pscr')
                if u == 0:
                    nc.vector.tensor_scalar(
                        out=pscr[:, 0:hf], in0=q[:, 0:hf],
                        scalar1=1.0, scalar2=0.0, op0=mybir.AluOpType.mult,
                        op1=mybir.AluOpType.add,
                        accum_out=pooled[:, u:u + 1])
                else:
                    nc.scalar.activation(
                        out=pscr[:, 0:hf], in_=q[:, 0:hf],
                        func=mybir.ActivationFunctionType.Copy,
                        accum_out=pooled[:, u:u + 1])
            att_ps = psum_att.tile([KEXP, 2], F32, tag='att_ps')
            att_s = smalls.tile([KEXP, 2], F32, tag='att_s', name=f'atts_{p}')
            for u in range(2):
                nc.tensor.matmul(att_ps[:, u:u + 1], lhsT=att_w_sb[:, bi, :],
                                 rhs=pooled[:, u:u + 1],
                                 start=True, stop=True)
                nc.scalar.activation(out=att_s[:, u:u + 1],
                                     in_=att_ps[:, u:u + 1],
                                     func=mybir.ActivationFunctionType.Sigmoid,
                                     bias=att_b_sb[:, bi:bi + 1])
            att_f = smalls.tile([1, 2 * KEXP], F32, tag='att_f',
                                name=f'attf_{p}')
            nc.scalar.dma_start(out=att_f, in_=att_s)  # (k,u) -> flat k*2+u
            att_bc = smalls.tile([128, 2 * KEXP], F32, tag='att_bc',
                                 name=f'attbc_{p}')
            nc.gpsimd.partition_broadcast(att_bc, att_f)
            return att_bc

        def aggregate(p, att_bc):
            bi, sp = PAIRS[p]
            ncol = len(BR[bi][3]) + len(BR[bi][4])
            aggs = []
            for u in range(2):
                agg = aggp.tile([128, ncol * C], BF16, tag='agg',
                                name=f'agg_{p}_{u}')
                nc.vector.tensor_scalar_mul(
                    out=agg, in0=w_sb[bi][:, 0],
                    scalar1=att_bc[:, u:u + 1])
                for k in range(1, KEXP):
                    nc.vector.scalar_tensor_tensor(
                        out=agg, in0=w_sb[bi][:, k],
                        scalar=att_bc[:, 2 * k + u:2 * k + u + 1],
                        in1=agg, op0=mybir.AluOpType.mult,
                        op1=mybir.AluOpType.add)
                aggs.append(agg)
            return aggs

        def conv_pair(p, aggs):
            """col-tiled conv for both units; returns psum tiles per bank."""
            bi, sp = PAIRS[p]
            bn, (ph, pw), shift, pairs, singles = BR[bi]
            cols = _col_taps(bi)
            slots = BR_SLOTS[bn]
            flat = ph * pw
            its = [in_tiles[(p, u)][:, 0:flat].rearrange('c (r q) -> c r q',
                                                         q=pw)
                   for u in range(2)]
            pts = [psum_conv.tile([128, NT], F32, tag='pt',
                                  name=f'pt_{p}_{t}') for t in range(N_TILES)]
            nslot = len(slots)
            for si, slot in enumerate(slots):
                first = (si == 0)
                last = (si == nslot - 1)
                for t in range(N_TILES):
                    r0 = RT * t
                    for u in range(2):
                        p0 = 64 * u
                        pt_u = pts[t][p0:p0 + 64, :]
                        agg = aggs[u]
                        it3 = its[u]
                        for jj, j in enumerate(slot):
                            kind, (dy, dx), half = cols[j]
                            st = first and jj == 0
                            sp_ = last and jj == len(slot) - 1
                            if kind == 'pair':
                                rhs = it3[:, r0 + dy:r0 + dy + RT, dx:dx + W]
                                nc.tensor.matmul(
                                    pt_u, lhsT=agg[:, j * C:(j + 1) * C],
                                    rhs=rhs, start=st, stop=sp_,
                                    skip_group_check=True)
                            else:
                                if half == 'lo':
                                    rhs = it3[0:64, r0 + dy:r0 + dy + RT,
                                              dx:dx + W]
                                    lhsT = agg[0:64, j * C:(j + 1) * C]
                                else:
                                    # shifted copy: flat idx - shift
                                    fo = dy * pw + dx - shift
                                    dy2, dx2 = fo // pw, fo % pw
                                    rhs = it3[64:128, r0 + dy2:r0 + dy2 + RT,
                                              dx2:dx2 + W]
                                    lhsT = agg[64:128, j * C:(j + 1) * C]
                                nc.tensor.matmul(
                                    pt_u, lhsT=lhsT, rhs=rhs, start=st,
                                    stop=sp_, skip_group_check=True)
            return pts

        def evac_stats(p, pts):
            """ACT evacuation (+Sum x), DVE Sum x^2, stage stats."""
            bi, sp = PAIRS[p]
            otile = out_tiles[p]
            for t in range(N_TILES):
                nc.scalar.activation(
                    out=otile[:, t * NT:(t + 1) * NT], in_=pts[t],
                    func=mybir.ActivationFunctionType.Copy,
                    accum_out=sx_t[p][:, t:t + 1])
            for t in range(N_TILES):
                sqs = sqscrp.tile([128, NT], BF16, tag='sqs')
                nc.vector.scalar_tensor_tensor(
                    out=sqs, in0=otile[:, t * NT:(t + 1) * NT], scalar=1.0,
                    in1=otile[:, t * NT:(t + 1) * NT],
                    op0=mybir.AluOpType.mult, op1=mybir.AluOpType.mult,
                    accum_out=sxx_t[p][:, t:t + 1])
            nc.vector.tensor_reduce(out=stage[:, bi, sp, 0:1], in_=sx_t[p],
                                    axis=mybir.AxisListType.X,
                                    op=mybir.AluOpType.add)
            nc.vector.tensor_reduce(out=stage[:, bi, sp, 1:2], in_=sxx_t[p],
                                    axis=mybir.AxisListType.X,
                                    op=mybir.AluOpType.add)

        def stage_collective():
            # local pair-sum + partition-half fold -> [64, 6], one AllReduce
            ps = persist.tile([128, 3, 2], F32, tag='ps', name='ps')
            nc.vector.tensor_reduce(
                out=ps, in_=stage.rearrange('c b p st -> c b st p'),
                axis=mybir.AxisListType.X, op=mybir.AluOpType.add)
            hi = persist.tile([64, 3, 2], F32, tag='hi', name='hi')
            nc.sync.dma_start(out=hi, in_=ps[64:128])
            half = persist.tile([64, 3, 2], F32, tag='half', name='half')
            nc.vector.tensor_tensor(out=half, in0=ps[0:64], in1=hi,
                                    op=mybir.AluOpType.add)
            nc.sync.dma_start(out=cc_in[0],
                                in_=half.rearrange('c b st -> c (b st)'))
            nc.gpsimd.collective_compute(
                'AllReduce', mybir.AluOpType.add,
                replica_groups=[list(range(N_CORES))],
                ins=[cc_in[0].opt()], outs=[cc_out[0].opt()])
            nc.sync.dma_start(
                out=g_sb[0],
                in_=cc_out[0].rearrange('c (b st) -> c b st', b=3))

        # ---------- pipeline ----------
        load_pair(0)
        load_pair(1)
        load_pair(2)
        aggs = aggregate(0, pool_att(0))
        pend = {0: aggs}
        for p in range(6):
            if p + 3 < 6:
                load_pair(p + 3)
            if p + 1 < 6:
                att_bc = pool_att(p + 1)
            pts = conv_pair(p, pend.pop(p))
            if p + 1 < 6:
                pend[p + 1] = aggregate(p + 1, att_bc)
            evac_stats(p, pts)
        stage_collective()

        # ---------- scale/bias ----------
        tot2 = g_sb[0]
        mv = persist.tile([C, 3, 2], F32, tag='mv')
        nc.vector.tensor_scalar_mul(out=mv, in0=tot2, scalar1=1.0 / M_TOTAL)
        var = persist.tile([C, 3], F32, tag='var')
        nc.vector.tensor_tensor(out=var, in0=mv[:, :, 0], in1=mv[:, :, 0],
                                op=mybir.AluOpType.mult)
        nc.vector.tensor_tensor(out=var, in0=mv[:, :, 1], in1=var,
                                op=mybir.AluOpType.subtract)
        sd = persist.tile([C, 3], F32, tag='sd')
        epst = persist.tile([C, 1], F32, tag='epst')
        nc.vector.memset(epst, EPS)
        nc.scalar.activation(out=sd, in_=var,
                             func=mybir.ActivationFunctionType.Sqrt, bias=epst)
        nc.vector.reciprocal(out=sd, in_=sd)
        scale2 = persist.tile([128, 3], F32, tag='scale2')
        bias2 = persist.tile([128, 3], F32, tag='bias2')
        nc.vector.tensor_tensor(out=scale2[0:64], in0=gb_sb[:, 0], in1=sd,
                                op=mybir.AluOpType.mult)
        tmpb = persist.tile([C, 3], F32, tag='tmpb')
        nc.vector.tensor_tensor(out=tmpb, in0=mv[:, :, 0], in1=scale2[0:64],
                                op=mybir.AluOpType.mult)
        nc.vector.tensor_tensor(out=bias2[0:64], in0=gb_sb[:, 1], in1=tmpb,
                                op=mybir.AluOpType.subtract)
        nc.sync.dma_start(out=scale2[64:128], in_=scale2[0:64])
        nc.sync.dma_start(out=bias2[64:128], in_=bias2[0:64])

        # ---------- normalize (ACT/DVE alternating) + stores ----------
        store_engines = [nc.sync, nc.scalar] * 6
        se = 0
        for p in range(6):
            bi, sp = PAIRS[p]
            otile = out_tiles[p]
            bounce = bouncep.tile([128, HW], F32, tag='bounce',
                                  name=f'bounce_{p}')
            if p % 2 == 0:
                nc.scalar.activation(out=bounce, in_=otile,
                                     func=mybir.ActivationFunctionType.Identity,
                                     bias=bias2[:, bi:bi + 1],
                                     scale=scale2[:, bi:bi + 1])
            else:
                nc.vector.tensor_scalar(
                    out=bounce, in0=otile,
                    scalar1=scale2[:, bi:bi + 1], scalar2=bias2[:, bi:bi + 1],
                    op0=mybir.AluOpType.mult, op1=mybir.AluOpType.add)
            for u in range(2):
                s = 2 * sp + u
                store_engines[se].dma_start(
                    out=ov[s, 2 * bi:2 * bi + 2],
                    in_=bounce[64 * u:64 * u + 64])
                se += 1


_NC_CACHE = None


def _get_nc():
    global _NC_CACHE
    if _NC_CACHE is None:
        _NC_CACHE = _build_nc()
    return _NC_CACHE


def _host_weights(w, bi):
    """w [K, O, Cin, kh, kw] -> [128, K, ncol*64] bf16 lhsT layout."""
    bn, (ph, pw), shift, pairs, singles = BR[bi]
    k, o, cin, kh, kw = w.shape
    ncol = len(pairs) + len(singles)
    wt = np.zeros((k, 128, ncol * C), np.float32)
    # kernel-tap (dy_k, dx_k) indices from padded-image tap (dy, dx):
    # conv output (y, x) tile row r0 reads padded rows r0+dy; the tap with
    # window offset (dy, dx) corresponds to kernel index (dy, dx) directly.
    for j, (dy, dx) in enumerate(pairs):
        # lower: tap (dy, dx); upper: flat+shift
        fo = dy * pw + dx + shift
        dy1, dx1 = fo // pw, fo % pw
        wt[:, 0:64, j * C:(j + 1) * C] = w[:, :, :, dy, dx].transpose(0, 2, 1)
        wt[:, 64:128, j * C:(j + 1) * C] = \
            w[:, :, :, dy1, dx1].transpose(0, 2, 1)
    npair = len(pairs)
    for j, (dy, dx, half) in enumerate(singles):
        blk = slice((npair + j) * C, (npair + j + 1) * C)
        tgt = slice(0, 64) if half == 'lo' else slice(64, 128)
        wt[:, tgt, blk] = w[:, :, :, dy, dx].transpose(0, 2, 1)
    return np.ascontiguousarray(
        wt.transpose(1, 0, 2)).astype(ml_dtypes.bfloat16)


def _prep_in_maps(inputs):
    x = np.ascontiguousarray(inputs['x'], dtype=np.float32)
    n_total = x.shape[0]
    pads = [(1, 1), (1, 0), (0, 1)]
    xpad = []
    for bi, (bn, (ph, pw), shift, pairs, singles) in enumerate(BR):
        ph_, pw_ = pads[bi]
        sl = x[:, C * (bi + 1):C * (bi + 2)]
        p = np.zeros((n_total, C, ph * pw + ROW_SLACK), ml_dtypes.bfloat16)
        img = p[:, :, :ph * pw].reshape(n_total, C, ph, pw)
        img[:, :, ph_:ph_ + H, pw_:pw_ + W] = sl.astype(ml_dtypes.bfloat16)
        xpad.append(np.ascontiguousarray(p))

    shared = {}
    names = [('sq', 'w_sq', 'att_w_sq', 'att_b_sq', 'g_sq', 'b_sq'),
             ('v', 'w_v', 'att_w_v', 'att_b_v', 'g_v', 'b_v'),
             ('h', 'w_h', 'att_w_h', 'att_b_h', 'g_h', 'b_h')]
    att_w_all = np.zeros((128, 3, KEXP), np.float32)
    att_b_all = np.zeros((KEXP, 3), np.float32)
    gb_all = np.zeros((C, 2, 3), np.float32)
    for bi, (bn, wk, awk, abk, gk, bk) in enumerate(names):
        w = np.asarray(inputs[wk], dtype=np.float32)
        # reference conv kernels for v ([3,1]) and h ([1,3]) index (kh, kw)
        kh, kw = w.shape[3], w.shape[4]
        wfull = np.zeros((KEXP, C, C, *_br_kshape(bi)), np.float32)
        wfull[:, :, :, :kh, :kw] = w
        shared[f'w_{bn}'] = _host_weights(wfull, bi)
        aw = np.asarray(inputs[awk], np.float32).T / float(HW)
        att_w_all[0:64, bi, :] = aw
        att_w_all[64:128, bi, :] = aw
        att_b_all[:, bi] = np.asarray(inputs[abk], np.float32)
        gb_all[:, 0, bi] = np.asarray(inputs[gk], np.float32)
        gb_all[:, 1, bi] = np.asarray(inputs[bk], np.float32)
    shared['att_w'] = att_w_all
    shared['att_b'] = att_b_all
    shared['gb'] = gb_all

    in_maps = []
    for ci in range(N_CORES):
        m = dict(shared)
        sl = slice(ci * NS, (ci + 1) * NS)
        for bi, (bn, _, _, _, _) in enumerate(BR):
            m[f'xp_{bn}'] = xpad[bi][sl]
        in_maps.append(m)
    return in_maps


def _br_kshape(bi):
    return [(3, 3), (3, 1), (1, 3)][bi]


def run_raw(inputs, trace=False, **kwargs):
    """Build+run; returns (full_output, BassKernelResults)."""
    nc = _get_nc()
    in_maps = _prep_in_maps(inputs)
    res = bass_utils.run_bass_kernel_spmd(
        nc, in_maps, core_ids=list(range(N_CORES)), trace=trace, **kwargs)
    dev = np.concatenate([res.results[i]['out'] for i in range(N_CORES)],
                         axis=0)                      # [32, 6, 32, HW]
    x = np.asarray(inputs['x'], dtype=np.float32)
    full = np.empty((32, 256, H, W), np.float32)
    o5 = full.reshape(32, 32, 8, H, W)
    # channel shuffle: shuffled[c2*8+g] = concat[g*32+c2]; s0 = concat[0:64]
    o5[:, :, 0] = x[:, 0:32]
    o5[:, :, 1] = x[:, 32:64]
    o5[:, :, 2:8] = dev.reshape(32, 6, 32, H, W).transpose(0, 2, 1, 3, 4)
    return full, res


def kernel(**inputs):
    full, _ = run_raw(inputs)
    return full
